# revision 7
# baseline (speedup 1.0000x reference)
"""MinGRU (B=4, T=4096, D=1024) Trainium2 kernel, 8-core SPMD.

Sharding: core i handles (batch b = i//2, output-channel half j = i%2).
Each core computes u_z = x[b] @ Wz[half].T, u_h = x[b] @ Wh[half].T,
z = sigmoid(u_z + bz), a = 1 - z, bvec = z * (u_h + bh), then the
recurrence h_t = a_t * h_{t-1} + b_t via the hardware tensor_tensor_scan.

Matmuls are fp32r (FP22 self-loading; no separate LDWEIGHTS, so the PE
stream is gap-free). x lives fully resident in SBUF (16 MiB = 128 KB per
partition, loaded once at startup), so the steady-state loop does no
input DMA at all and the only per-iteration DMA is the bf16 h output
(4 MiB). Epilogue keeps one PSUM reader per bank: ACT does
z = sigmoid(pz + bz); DVE does a = 1 - z, b = (ph + bh) * z, and the
scan (fp32 state, bf16 out). Host pre-transposes x and W so every DMA is
row-contiguous and converts h back to fp32.

The sustained per-iteration time is pinned by the PE clock governor
(K=4/8 under sustained 8-core load): ~159 us = 262144 streamed columns
at ~1.65 GHz. This version minimizes everything above that floor and
keeps the PE stream dense enough to catch the full-clock state when the
governor allows.
"""

import numpy as np

_B, _T, _D = 4, 4096, 1024
_EH = 512
_NG = _EH // 128
_TT = 512
_NT = _T // _TT    # 8 resident t-tiles
_NK = _D // 128


def _build(reps=1, loop_n=None):
    from contextlib import ExitStack
    from concourse import bacc, mybir, tile

    f32 = mybir.dt.float32
    f32r = mybir.dt.float32r
    bf16 = mybir.dt.bfloat16
    AF = mybir.ActivationFunctionType
    OP = mybir.AluOpType

    nc = bacc.Bacc("TRN2", debug=False, num_devices=8)
    xt = nc.dram_tensor("xt", [_D, _T], f32, kind="ExternalInput").ap()
    wzt = nc.dram_tensor("wzt", [_D, _EH], f32, kind="ExternalInput").ap()
    wht = nc.dram_tensor("wht", [_D, _EH], f32, kind="ExternalInput").ap()
    bzt = nc.dram_tensor("bzt", [128, _NG], f32, kind="ExternalInput").ap()
    bht = nc.dram_tensor("bht", [128, _NG], f32, kind="ExternalInput").ap()
    hout = nc.dram_tensor("h", [_EH, _T], bf16, kind="ExternalOutput").ap()

    with tile.TileContext(nc) as tc, ExitStack() as ctx:
        wpool = ctx.enter_context(tc.tile_pool(name="w", bufs=1))
        vpool = ctx.enter_context(tc.tile_pool(name="v", bufs=3))
        hpool = ctx.enter_context(tc.tile_pool(name="h", bufs=2))
        ppool = ctx.enter_context(tc.tile_pool(name="p", bufs=3, space="PSUM"))
        plast = ctx.enter_context(tc.tile_pool(name="pl", bufs=1, space="PSUM"))

        # x fully resident: 8 t-tiles of [128, (k tt)] fp32r = 128 KB/partition.
        xres = []
        for t in range(_NT):
            xres_t = wpool.tile([128, _NK * _TT], f32r, tag=f"x{t}")
            xres.append(xres_t)
        wz_sb = wpool.tile([128, _NK * _EH], f32r, tag="wz")
        wh_sb = wpool.tile([128, _NK * _EH], f32r, tag="wh")
        bz_sb = wpool.tile([128, _NG], f32, tag="bz")
        bh_sb = wpool.tile([128, _NG], f32, tag="bh")

        def x_chunk(t, ks, nk):
            nc.sync.dma_start(
                xres[t][:, ks * _TT:(ks + nk) * _TT].rearrange(
                    "p (k t) -> p k t", k=nk),
                xt.rearrange("(k p) t -> p k t", p=128)[
                    :, ks:ks + nk, t * _TT:(t + 1) * _TT].bitcast(f32r),
            )

        def w_chunk(k):
            nc.sync.dma_start(
                wz_sb[:, k * _EH:(k + 1) * _EH],
                wzt[k * 128:(k + 1) * 128, :].bitcast(f32r))
            nc.sync.dma_start(
                wh_sb[:, k * _EH:(k + 1) * _EH],
                wht[k * 128:(k + 1) * 128, :].bitcast(f32r))

        x_chunk(0, 0, 4)
        w_chunk(0)
        w_chunk(1)
        nc.sync.dma_start(bz_sb[:], bzt)
        nc.sync.dma_start(bh_sb[:], bht)
        x_chunk(0, 4, 4)
        for k in range(2, _NK):
            w_chunk(k)
        for t in range(1, _NT):
            x_chunk(t, 0, 4)
            x_chunk(t, 4, 4)

        def body(first):
          hprev = [None] * _NG
          for t in range(_NT):
            xs = xres[t]
            for g in range(_NG):
                last = (t == _NT - 1 and g == _NG - 1)
                halves = ((0, _TT // 2), (_TT // 2, _TT // 2)) if last \
                    else ((0, _TT),)
                prev_ap = None if t == 0 else hprev[g][:, _TT - 1:_TT]
                for (c0, w) in halves:
                    pool = plast if last else ppool
                    pz = pool.tile([128, w], f32, tag="pzl" if last else "pz")
                    ph = pool.tile([128, w], f32, tag="phl" if last else "ph")
                    for k in range(_NK):
                        nc.tensor.matmul(
                            pz[:],
                            lhsT=wz_sb[:, k * _EH + g * 128:
                                       k * _EH + (g + 1) * 128],
                            rhs=xs[:, k * _TT + c0: k * _TT + c0 + w],
                            start=(k == 0),
                            stop=(k == _NK - 1),
                        )
                    for k in range(_NK):
                        nc.tensor.matmul(
                            ph[:],
                            lhsT=wh_sb[:, k * _EH + g * 128:
                                       k * _EH + (g + 1) * 128],
                            rhs=xs[:, k * _TT + c0: k * _TT + c0 + w],
                            start=(k == 0),
                            stop=(k == _NK - 1),
                        )
                    z = vpool.tile([128, w], f32, tag="z")
                    nc.scalar.activation(z[:], pz[:], AF.Sigmoid,
                                         bias=bz_sb[:, g:g + 1])
                    av = vpool.tile([128, w], f32, tag="a")
                    nc.scalar.activation(av[:], z[:], AF.Copy,
                                         bias=1.0, scale=-1.0)
                    bv = vpool.tile([128, w], f32, tag="b")
                    nc.vector.scalar_tensor_tensor(
                        bv[:], ph[:], bh_sb[:, g:g + 1], z[:],
                        OP.add, OP.mult)
                    hb = hpool.tile([128, w], bf16, tag=f"h{g}")
                    init = 0.0 if prev_ap is None else prev_ap
                    nc.vector.tensor_tensor_scan(hb[:], av[:], bv[:], init,
                                                 OP.mult, OP.add)
                    prev_ap = hb[:, w - 1:w]
                    if not last:
                        hprev[g] = hb
                    nc.sync.dma_start(
                        hout[g * 128:(g + 1) * 128,
                             t * _TT + c0: t * _TT + c0 + w], hb[:])

        if loop_n is not None:
            body(True)
            from concourse import mybir as _mb
            with tc.For_i(0, loop_n, 1, hint_engines=(
                    _mb.EngineType.PE, _mb.EngineType.SP,
                    _mb.EngineType.DVE, _mb.EngineType.Activation),
                    staggered_reset=True):
                body(False)
        else:
            for rep in range(reps):
                body(rep == 0)
    nc.compile()
    return nc


_NC_CACHE = None


def _shard_inputs(inputs):
    x = np.asarray(inputs["x"], dtype=np.float32)
    Wz = np.asarray(inputs["Wz"], dtype=np.float32)
    bz = np.asarray(inputs["bz"], dtype=np.float32)
    Wh = np.asarray(inputs["Wh"], dtype=np.float32)
    bh = np.asarray(inputs["bh"], dtype=np.float32)

    wzT = np.ascontiguousarray(Wz.T)
    whT = np.ascontiguousarray(Wh.T)

    in_maps = []
    for i in range(8):
        b, j = i // 2, i % 2
        sl = slice(j * _EH, (j + 1) * _EH)
        in_maps.append({
            "xt": np.ascontiguousarray(x[b].T),
            "wzt": np.ascontiguousarray(wzT[:, sl]),
            "wht": np.ascontiguousarray(whT[:, sl]),
            "bzt": np.ascontiguousarray(bz[sl].reshape(_NG, 128).T),
            "bht": np.ascontiguousarray(bh[sl].reshape(_NG, 128).T),
        })
    return in_maps


def run(inputs, trace=False, tmpdir=None):
    global _NC_CACHE
    from concourse.bass_utils import run_bass_kernel_spmd

    if _NC_CACHE is None:
        _NC_CACHE = _build()
    nc = _NC_CACHE
    in_maps = _shard_inputs(inputs)
    res = run_bass_kernel_spmd(
        nc, in_maps, core_ids=list(range(8)), trace=trace, tmpdir=tmpdir)
    out = np.empty((_B, _T, _D), dtype=np.float32)
    for i in range(8):
        b, j = i // 2, i % 2
        out[b, :, j * _EH:(j + 1) * _EH] = res.results[i]["h"].astype(np.float32).T
    return out, res


def kernel(**inputs):
    out, _ = run(inputs, trace=False)
    return out


# revision 8
# speedup vs baseline: 1.0183x; 1.0183x over previous
"""MinGRU (B=4, T=4096, D=1024) Trainium2 kernel, 8-core SPMD.

Sharding: core i handles (batch b = i//2, output-channel half j = i%2).
Each core computes u_z = x[b] @ Wz[half].T, u_h = x[b] @ Wh[half].T,
z = sigmoid(u_z + bz), a = 1 - z, bvec = z * (u_h + bh), then the
recurrence h_t = a_t * h_{t-1} + b_t via the hardware tensor_tensor_scan.

Matmuls are fp32r (FP22 self-loading; no separate LDWEIGHTS, so the PE
stream is gap-free). x lives fully resident in SBUF (16 MiB = 128 KB per
partition, loaded once at startup), so the steady-state loop does no
input DMA at all and the only per-iteration DMA is the bf16 h output
(4 MiB). Epilogue keeps one PSUM reader per bank: ACT does
z = sigmoid(pz + bz); DVE does a = 1 - z, b = (ph + bh) * z, and the
scan (fp32 state, bf16 out). Host pre-transposes x and W so every DMA is
row-contiguous and converts h back to fp32.

The sustained per-iteration time is pinned by the PE clock governor
(K=4/8 under sustained 8-core load): ~159 us = 262144 streamed columns
at ~1.65 GHz. This version minimizes everything above that floor and
keeps the PE stream dense enough to catch the full-clock state when the
governor allows.
"""

import numpy as np

_B, _T, _D = 4, 4096, 1024
_EH = 512
_NG = _EH // 128
_TT = 512
_NT = _T // _TT    # 8 resident t-tiles
_NK = _D // 128


def _build(reps=1, loop_n=None):
    from contextlib import ExitStack
    from concourse import bacc, mybir, tile

    f32 = mybir.dt.float32
    f32r = mybir.dt.float32r
    bf16 = mybir.dt.bfloat16
    AF = mybir.ActivationFunctionType
    OP = mybir.AluOpType

    nc = bacc.Bacc("TRN2", debug=False, num_devices=8)
    xt = nc.dram_tensor("xt", [_D, _T], f32, kind="ExternalInput").ap()
    wzt = nc.dram_tensor("wzt", [_D, _EH], f32, kind="ExternalInput").ap()
    wht = nc.dram_tensor("wht", [_D, _EH], f32, kind="ExternalInput").ap()
    bzt = nc.dram_tensor("bzt", [128, _NG], f32, kind="ExternalInput").ap()
    bht = nc.dram_tensor("bht", [128, _NG], f32, kind="ExternalInput").ap()
    hout = nc.dram_tensor("h", [_EH, _T], bf16, kind="ExternalOutput").ap()

    with tile.TileContext(nc) as tc, ExitStack() as ctx:
        wpool = ctx.enter_context(tc.tile_pool(name="w", bufs=1))
        vpool = ctx.enter_context(tc.tile_pool(name="v", bufs=3))
        hpool = ctx.enter_context(tc.tile_pool(name="h", bufs=2))
        ppool = ctx.enter_context(tc.tile_pool(name="p", bufs=3, space="PSUM"))
        plast = ctx.enter_context(tc.tile_pool(name="pl", bufs=1, space="PSUM"))

        # x fully resident: 8 t-tiles of [128, (k tt)] fp32r = 128 KB/partition.
        xres = []
        for t in range(_NT):
            xres_t = wpool.tile([128, _NK * _TT], f32r, tag=f"x{t}")
            xres.append(xres_t)
        wz_sb = wpool.tile([128, _NK * _EH], f32r, tag="wz")
        wh_sb = wpool.tile([128, _NK * _EH], f32r, tag="wh")
        bz_sb = wpool.tile([128, _NG], f32, tag="bz")
        bh_sb = wpool.tile([128, _NG], f32, tag="bh")

        def x_chunk(t, ks, nk):
            nc.sync.dma_start(
                xres[t][:, ks * _TT:(ks + nk) * _TT].rearrange(
                    "p (k t) -> p k t", k=nk),
                xt.rearrange("(k p) t -> p k t", p=128)[
                    :, ks:ks + nk, t * _TT:(t + 1) * _TT].bitcast(f32r),
            )

        def w_chunk(k):
            nc.sync.dma_start(
                wz_sb[:, k * _EH:(k + 1) * _EH],
                wzt[k * 128:(k + 1) * 128, :].bitcast(f32r))
            nc.sync.dma_start(
                wh_sb[:, k * _EH:(k + 1) * _EH],
                wht[k * 128:(k + 1) * 128, :].bitcast(f32r))

        x_chunk(0, 0, 4)
        w_chunk(0)
        w_chunk(1)
        nc.sync.dma_start(bz_sb[:], bzt)
        nc.sync.dma_start(bh_sb[:], bht)
        x_chunk(0, 4, 4)
        for k in range(2, _NK):
            w_chunk(k)
        for t in range(1, _NT):
            x_chunk(t, 0, 4)
            x_chunk(t, 4, 4)

        def body(first):
          hprev = [None] * _NG
          for t in range(_NT):
            xs = xres[t]
            for g in range(_NG):
                last = (t == _NT - 1 and g == _NG - 1)
                halves = ((0, _TT // 2), (_TT // 2, _TT // 2)) if last \
                    else ((0, _TT),)
                prev_ap = None if t == 0 else hprev[g][:, _TT - 1:_TT]
                for (c0, w) in halves:
                    pool = plast if last else ppool
                    pz = pool.tile([128, w], f32, tag="pzl" if last else "pz")
                    ph = pool.tile([128, w], f32, tag="phl" if last else "ph")
                    for k in range(_NK):
                        nc.tensor.matmul(
                            pz[:],
                            lhsT=wz_sb[:, k * _EH + g * 128:
                                       k * _EH + (g + 1) * 128],
                            rhs=xs[:, k * _TT + c0: k * _TT + c0 + w],
                            start=(k == 0),
                            stop=(k == _NK - 1),
                        )
                    for k in range(_NK):
                        nc.tensor.matmul(
                            ph[:],
                            lhsT=wh_sb[:, k * _EH + g * 128:
                                       k * _EH + (g + 1) * 128],
                            rhs=xs[:, k * _TT + c0: k * _TT + c0 + w],
                            start=(k == 0),
                            stop=(k == _NK - 1),
                        )
                    z = vpool.tile([128, w], f32, tag="z")
                    nc.scalar.activation(z[:], pz[:], AF.Sigmoid,
                                         bias=bz_sb[:, g:g + 1])
                    av = vpool.tile([128, w], f32, tag="a")
                    nc.vector.tensor_scalar(
                        av[:], z[:], -1.0, 1.0, OP.mult, OP.add)
                    bv = vpool.tile([128, w], f32, tag="b")
                    nc.vector.scalar_tensor_tensor(
                        bv[:], ph[:], bh_sb[:, g:g + 1], z[:],
                        OP.add, OP.mult)
                    hb = hpool.tile([128, w], bf16, tag=f"h{g}")
                    init = 0.0 if prev_ap is None else prev_ap
                    nc.vector.tensor_tensor_scan(hb[:], av[:], bv[:], init,
                                                 OP.mult, OP.add)
                    prev_ap = hb[:, w - 1:w]
                    if not last:
                        hprev[g] = hb
                    nc.sync.dma_start(
                        hout[g * 128:(g + 1) * 128,
                             t * _TT + c0: t * _TT + c0 + w], hb[:])

        if loop_n is not None:
            body(True)
            from concourse import mybir as _mb
            with tc.For_i(0, loop_n, 1, hint_engines=(
                    _mb.EngineType.PE, _mb.EngineType.SP,
                    _mb.EngineType.DVE, _mb.EngineType.Activation),
                    staggered_reset=True):
                body(False)
        else:
            for rep in range(reps):
                body(rep == 0)
    nc.compile()
    return nc


_NC_CACHE = None


def _shard_inputs(inputs):
    x = np.asarray(inputs["x"], dtype=np.float32)
    Wz = np.asarray(inputs["Wz"], dtype=np.float32)
    bz = np.asarray(inputs["bz"], dtype=np.float32)
    Wh = np.asarray(inputs["Wh"], dtype=np.float32)
    bh = np.asarray(inputs["bh"], dtype=np.float32)

    wzT = np.ascontiguousarray(Wz.T)
    whT = np.ascontiguousarray(Wh.T)

    in_maps = []
    for i in range(8):
        b, j = i // 2, i % 2
        sl = slice(j * _EH, (j + 1) * _EH)
        in_maps.append({
            "xt": np.ascontiguousarray(x[b].T),
            "wzt": np.ascontiguousarray(wzT[:, sl]),
            "wht": np.ascontiguousarray(whT[:, sl]),
            "bzt": np.ascontiguousarray(bz[sl].reshape(_NG, 128).T),
            "bht": np.ascontiguousarray(bh[sl].reshape(_NG, 128).T),
        })
    return in_maps


def run(inputs, trace=False, tmpdir=None):
    global _NC_CACHE
    from concourse.bass_utils import run_bass_kernel_spmd

    if _NC_CACHE is None:
        _NC_CACHE = _build()
    nc = _NC_CACHE
    in_maps = _shard_inputs(inputs)
    res = run_bass_kernel_spmd(
        nc, in_maps, core_ids=list(range(8)), trace=trace, tmpdir=tmpdir)
    out = np.empty((_B, _T, _D), dtype=np.float32)
    for i in range(8):
        b, j = i // 2, i % 2
        out[b, :, j * _EH:(j + 1) * _EH] = res.results[i]["h"].astype(np.float32).T
    return out, res


def kernel(**inputs):
    out, _ = run(inputs, trace=False)
    return out
